# revision 17
# baseline (speedup 1.0000x reference)
"""Trainium2 Bass kernel for nn_CausalSelfAttention_40810779247124.

Head-sharded (tensor-parallel) causal self-attention prefill across 8
NeuronCores: 2 heads per core. Per core:

  phase 1: QKV projection for its 2 heads, outputs in [e, tok] layout
           (contraction-friendly), Q/K kept resident in SBUF, V
           PE-transposed to [tok, e] and kept resident in SBUF.
  phase 2: attention computed transposed: scoresT[t,s] = K.T @ Q (both
           operands already have Dh on partitions), exp on ScalarE,
           denominator via ones-matmul (partition-sum on PE),
           wvT[Dh,s] = V.T @ P.T accumulated on PE.  wvT staged to DRAM.
  phase 3: output projection partial: out[tok,:] += wvT.T @ w_outT for
           this core's d-slice.  The all-reduce over cores is done on
           the host during unsharding (sum of 8 partials).

Causality is exploited (t>s blocks skipped); the host verifies that
mask/cache_pos match the causal-prefill pattern and falls back to a
numpy reference otherwise.  All matmul operands use float32r (full-rate
fp32 matmul mode on TRN2).
"""

import sys

sys.path.insert(0, "/opt/trn_rl_repo")

import numpy as np

B = 2
S = 2048
T = 4096
NS = 2048          # n_state
H = 16
DH = 128
NCORES = 8
HPC = H // NCORES  # heads per core = 2
DPC = HPC * DH     # d-slice per core = 256
TOK = B * S        # 4096 tokens across batches
SCALE = 1.0 / float(np.sqrt(DH))

_CACHED = {}


def _build_program():
    import concourse.bacc as bacc
    import concourse.tile as tile
    from concourse import mybir
    f32r = mybir.dt.float32r
    f32 = mybir.dt.float32

    nc = bacc.Bacc()

    xT = nc.dram_tensor("xT", [NS, TOK], f32r, kind="ExternalInput")
    wT = nc.dram_tensor("wT", [NS, 6 * DH], f32r, kind="ExternalInput")
    woutT = nc.dram_tensor("woutT", [DPC, NS], f32r, kind="ExternalInput")
    cmask = nc.dram_tensor("cmask", [DH, 4 * 512 + 256], f32r, kind="ExternalInput")
    outp = nc.dram_tensor("outp", [TOK, NS], f32, kind="ExternalOutput")

    NT = TOK // 512   # 8 tok-tiles of 512
    NK = NS // 128    # 16 contraction chunks

    with tile.TileContext(nc) as tc:
        with (
            tc.tile_pool(name="constp", bufs=1) as constp,
            tc.tile_pool(name="vresp", bufs=1) as vresp,
            tc.tile_pool(name="dramp", bufs=1, space="DRAM") as dramp,
        ):
            cmask_sb = constp.tile([DH, 4 * 512 + 256], f32r)
            nc.gpsimd.dma_start(out=cmask_sb, in_=cmask[:, :])
            identity = cmask_sb[:, 2048:2176]
            ones_col = cmask_sb[:, 2176:2177]
            ones_row = cmask_sb[0:1, 2176:2304]

            # V resident across phases 1-2: v_res[p, c, e] = V[c*128+p, e]
            v_res = vresp.tile([128, TOK // 128, DPC], f32r)

            # wvT staging through DRAM between phases 2 and 3
            wvn_d = dramp.tile([B * DPC, S], f32r)

            with tc.tile_pool(name="qkresp", bufs=1) as qkresp:
                # Q,K resident [e-block(q0,q1,k0,k1), tok]
                qk_res = qkresp.tile([128, 4, TOK], f32r)

                # ---------------- phase 1: QKV projection ----------------
                with (
                    tc.tile_pool(name="wp", bufs=1) as wp,
                    tc.tile_pool(name="xp", bufs=3) as xp,
                    tc.tile_pool(name="vstage", bufs=3) as vstage,
                    tc.tile_pool(name="qkv_ps", bufs=6, space="PSUM") as qkv_ps,
                    tc.tile_pool(name="tr_ps", bufs=2, space="PSUM") as tr_ps,
                ):
                    w_sb = wp.tile([128, NK, 6 * DH], f32r)
                    nc.gpsimd.dma_start(
                        out=w_sb, in_=wT.rearrange("(c p) e -> p c e", p=128)
                    )

                    for a in range(NT):
                        pss = [
                            qkv_ps.tile([128, 512], f32, tag="qkv", name=f"qkv{m}")
                            for m in range(6)
                        ]
                        for half in range(2):
                            x_sb = xp.tile([128, NK // 2, 512], f32r, tag="x_sb")
                            nc.gpsimd.dma_start(
                                out=x_sb,
                                in_=xT[
                                    1024 * half : 1024 * (half + 1),
                                    512 * a : 512 * (a + 1),
                                ].rearrange("(c p) t -> p c t", p=128),
                            )
                            for m in range(6):
                                for kc in range(NK // 2):
                                    kk = half * (NK // 2) + kc
                                    nc.tensor.matmul(
                                        pss[m],
                                        w_sb[:, kk, 128 * m : 128 * (m + 1)],
                                        x_sb[:, kc, :],
                                        start=(kk == 0),
                                        stop=(kk == NK - 1),
                                    )
                        for m in range(4):
                            # Q/K to resident SBUF in [e, tok] layout
                            nc.vector.tensor_copy(
                                out=qk_res[:, m, 512 * a : 512 * (a + 1)],
                                in_=pss[m],
                            )
                        for h in range(HPC):
                            # V: transpose [e,tok] -> [tok,e] into v_res
                            vs = vstage.tile([128, 512], f32r, tag="v")
                            nc.vector.tensor_copy(out=vs, in_=pss[4 + h])
                            for t in range(4):
                                tp = tr_ps.tile([128, 128], f32r, tag="tp")
                                nc.tensor.transpose(
                                    tp, vs[:, 128 * t : 128 * (t + 1)], identity
                                )
                                nc.vector.tensor_copy(
                                    out=v_res[
                                        :, 4 * a + t, 128 * h : 128 * (h + 1)
                                    ],
                                    in_=tp,
                                )

                # ---------------- phase 2: attention ----------------
                with (
                    tc.tile_pool(name="ptp", bufs=4) as ptp,
                    tc.tile_pool(name="zrp", bufs=2) as zrp,
                    tc.tile_pool(name="wvnp", bufs=2) as wvnp,
                    tc.tile_pool(name="sc_ps", bufs=2, space="PSUM") as sc_ps,
                    tc.tile_pool(name="wv_ps", bufs=2, space="PSUM") as wv_ps,
                    tc.tile_pool(name="z_ps", bufs=2, space="PSUM") as z_ps,
                    tc.tile_pool(name="zb_ps", bufs=2, space="PSUM") as zb_ps,
                ):
                    for b in range(B):
                        for h in range(HPC):
                            q_sb = qk_res[:, h, S * b : S * (b + 1)]
                            k_sb = qk_res[:, 2 + h, S * b : S * (b + 1)]
                            wvn = wvnp.tile([128, S], f32r, tag="wvn")
                            for ast in range(S // 512):
                                nj = 4 * ast + 4  # causal t-blocks
                                wv = wv_ps.tile([128, 512], f32, tag="wv")
                                z = z_ps.tile([1, 512], f32, tag="z")
                                for j in range(nj):
                                    sc = sc_ps.tile([128, 512], f32, tag="sc")
                                    nc.tensor.matmul(
                                        sc,
                                        k_sb[:, 128 * j : 128 * (j + 1)],
                                        q_sb[:, 512 * ast : 512 * (ast + 1)],
                                        start=True,
                                        stop=True,
                                    )
                                    pt = ptp.tile([128, 512], f32r, tag="pt")
                                    nc.scalar.activation(
                                        out=pt,
                                        in_=sc,
                                        func=mybir.ActivationFunctionType.Exp,
                                        scale=SCALE,
                                    )
                                    p = j - 4 * ast
                                    if p >= 0:
                                        nc.vector.tensor_mul(
                                            pt,
                                            pt,
                                            cmask_sb[:, 512 * p : 512 * (p + 1)],
                                        )
                                    nc.tensor.matmul(
                                        z,
                                        ones_col,
                                        pt,
                                        start=(j == 0),
                                        stop=(j == nj - 1),
                                    )
                                    nc.tensor.matmul(
                                        wv,
                                        v_res[
                                            :, 16 * b + j, 128 * h : 128 * (h + 1)
                                        ],
                                        pt,
                                        start=(j == 0),
                                        stop=(j == nj - 1),
                                    )
                                zr = zrp.tile([1, 512], f32r, tag="zr")
                                with nc.allow_low_precision(
                                    reason="f32r is bit-identical to f32"
                                ):
                                    nc.vector.reciprocal(out=zr, in_=z)
                                zb = zb_ps.tile([128, 512], f32, tag="zb")
                                nc.tensor.matmul(
                                    zb, ones_row, zr, start=True, stop=True
                                )
                                zbs = zrp.tile([128, 512], f32r, tag="zbs")
                                nc.vector.tensor_copy(out=zbs, in_=zb)
                                nc.vector.tensor_mul(
                                    wvn[:, 512 * ast : 512 * (ast + 1)], wv, zbs
                                )
                            nc.gpsimd.dma_start(
                                out=wvn_d[
                                    (b * HPC + h) * DH : (b * HPC + h + 1) * DH, :
                                ],
                                in_=wvn,
                            )

            # ---------------- phase 3: output projection ----------------
            with (
                tc.tile_pool(name="woutp", bufs=1) as woutp,
                tc.tile_pool(name="wvlp", bufs=3) as wvlp,
                tc.tile_pool(name="ostage", bufs=3) as ostage,
                tc.tile_pool(name="o_ps", bufs=4, space="PSUM") as o_ps,
            ):
                wout_sb = woutp.tile([128, HPC, NS], f32r)
                nc.gpsimd.dma_start(
                    out=wout_sb, in_=woutT.rearrange("(c p) e -> p c e", p=128)
                )
                for b in range(B):
                    wvl = wvlp.tile([128, HPC, S], f32r, tag="wvl")
                    nc.gpsimd.dma_start(
                        out=wvl,
                        in_=wvn_d[b * DPC : (b + 1) * DPC, :].rearrange(
                            "(c p) t -> p c t", p=128
                        ),
                    )
                    for tk in range(S // 128):
                        ost = ostage.tile([128, NS], f32, tag="ost")
                        for n in range(NS // 512):
                            ops = o_ps.tile([128, 512], f32, tag="ops")
                            for h in range(HPC):
                                nc.tensor.matmul(
                                    ops,
                                    wvl[:, h, 128 * tk : 128 * (tk + 1)],
                                    wout_sb[:, h, 512 * n : 512 * (n + 1)],
                                    start=(h == 0),
                                    stop=(h == HPC - 1),
                                )
                            nc.vector.tensor_copy(
                                out=ost[:, 512 * n : 512 * (n + 1)], in_=ops
                            )
                        nc.gpsimd.dma_start(
                            out=outp[
                                S * b + 128 * tk : S * b + 128 * (tk + 1), :
                            ],
                            in_=ost,
                        )

    nc.compile()
    return nc


def _causal_fastpath_ok(mask, cache_pos):
    if cache_pos.shape != (S,) or not np.array_equal(
        np.asarray(cache_pos), np.arange(S, dtype=np.int64).astype(cache_pos.dtype)
    ):
        return False
    m = np.asarray(mask).reshape(S, T)
    rows = np.arange(S)[:, None]
    cols = np.arange(T)[None, :]
    return np.array_equal(m, cols <= rows)


def _numpy_fallback(input_ids, mask, cache_pos, w_qkv, w_out, k_cache, v_cache):
    x = np.asarray(input_ids, dtype=np.float32)
    qkv = np.einsum("bsd,ed->bse", x, np.asarray(w_qkv, np.float32))
    q, k, v = np.split(qkv, 3, axis=-1)

    def heads(t):
        return t.reshape(B, S, H, DH).transpose(0, 2, 1, 3)

    q, k, v = heads(q), heads(k), heads(v)
    kf = np.array(k_cache, np.float32)
    vf = np.array(v_cache, np.float32)
    kf[:, :, np.asarray(cache_pos)] = k
    vf[:, :, np.asarray(cache_pos)] = v
    sc = np.einsum("bhsd,bhtd->bhst", q, kf) * SCALE
    sc = np.where(np.asarray(mask), sc, np.finfo(np.float32).min)
    sc = sc - sc.max(axis=-1, keepdims=True)
    p = np.exp(sc)
    p = p / p.sum(axis=-1, keepdims=True)
    wv = np.einsum("bhst,bhtd->bhsd", p, vf)
    wv = wv.transpose(0, 2, 1, 3).reshape(B, S, NS)
    return np.einsum("bsd,ed->bse", wv, np.asarray(w_out, np.float32))


def _build_cmask_host():
    # 4 multiplicative mask tiles [128, 512] laid side by side: tile p is
    # applied to scoresT block (t rows) against an s-tile of width 512 when
    # the t-block is the p-th 128-strip inside that s-tile.
    t = np.arange(128)[:, None]
    s = np.arange(512)[None, :]
    tiles = []
    for p in range(4):
        tiles.append(((s - 128 * p) >= t).astype(np.float32))
    # trailing constant blocks: [identity(128) | ones(128)]
    tiles.append(np.eye(128, dtype=np.float32))
    tiles.append(np.ones((128, 128), dtype=np.float32))
    return np.concatenate(tiles, axis=1)  # [128, 2304]


def _run_on_device(in_maps, trace=False):
    from concourse.bass_utils import run_bass_kernel_spmd

    if "nc" not in _CACHED:
        _CACHED["nc"] = _build_program()
    nc = _CACHED["nc"]
    return run_bass_kernel_spmd(
        nc, in_maps, core_ids=list(range(NCORES)), trace=trace
    )


def _prep_in_maps(input_ids, w_qkv, w_out):
    x2d = np.ascontiguousarray(
        np.asarray(input_ids, np.float32).reshape(TOK, NS).T
    )  # [NS, TOK]
    cm = _build_cmask_host()
    wq = np.asarray(w_qkv, np.float32)
    wo = np.asarray(w_out, np.float32)
    in_maps = []
    for c in range(NCORES):
        lo, hi = c * DPC, (c + 1) * DPC
        w_slice = np.concatenate(
            [wq[lo:hi], wq[NS + lo : NS + hi], wq[2 * NS + lo : 2 * NS + hi]],
            axis=0,
        )  # [768, NS] (q,k,v rows for this core's heads)
        wT_c = np.ascontiguousarray(w_slice.T)        # [NS, 768]
        woutT_c = np.ascontiguousarray(wo[:, lo:hi].T)  # [DPC, NS]
        in_maps.append({"xT": x2d, "wT": wT_c, "woutT": woutT_c, "cmask": cm})
    return in_maps


def kernel(input_ids, mask, cache_pos, w_qkv, w_out, k_cache, v_cache):
    if not _causal_fastpath_ok(mask, cache_pos):
        return _numpy_fallback(
            input_ids, mask, cache_pos, w_qkv, w_out, k_cache, v_cache
        )
    in_maps = _prep_in_maps(input_ids, w_qkv, w_out)
    res = _run_on_device(in_maps)
    out = np.zeros((TOK, NS), np.float32)
    for r in res.results:
        out += r["outp"]
    return out.reshape(B, S, NS)


# revision 18
# speedup vs baseline: 1.6149x; 1.6149x over previous
"""Trainium2 Bass kernel for nn_CausalSelfAttention_40810779247124.

Head-sharded (tensor-parallel) causal self-attention prefill across 8
NeuronCores: 2 heads per core. Per core:

  phase 1: QKV projection for its 2 heads, outputs in [e, tok] layout
           (contraction-friendly), Q/K kept resident in SBUF, V
           PE-transposed to [tok, e] and kept resident in SBUF.
  phase 2: attention computed transposed: scoresT[t,s] = K.T @ Q (both
           operands already have Dh on partitions), exp on ScalarE,
           denominator via ones-matmul (partition-sum on PE),
           wvT[Dh,s] = V.T @ P.T accumulated on PE.  wvT staged to DRAM.
  phase 3: output projection partial: out[tok,:] += wvT.T @ w_outT for
           this core's d-slice.  The all-reduce over cores is done on
           the host during unsharding (sum of 8 partials).

Causality is exploited (t>s blocks skipped); the host verifies that
mask/cache_pos match the causal-prefill pattern and falls back to a
numpy reference otherwise.  All matmul operands use float32r (full-rate
fp32 matmul mode on TRN2).
"""

import sys

sys.path.insert(0, "/opt/trn_rl_repo")

import numpy as np

B = 2
S = 2048
T = 4096
NS = 2048          # n_state
H = 16
DH = 128
NCORES = 8
HPC = H // NCORES  # heads per core = 2
DPC = HPC * DH     # d-slice per core = 256
TOK = B * S        # 4096 tokens across batches
SCALE = 1.0 / float(np.sqrt(DH))

_CACHED = {}


def _build_program():
    import concourse.bacc as bacc
    import concourse.tile as tile
    from concourse import mybir
    f32r = mybir.dt.float32r
    f32 = mybir.dt.float32

    nc = bacc.Bacc()

    xT = nc.dram_tensor("xT", [NS, TOK], f32r, kind="ExternalInput")
    wT = nc.dram_tensor("wT", [NS, 6 * DH], f32r, kind="ExternalInput")
    woutT = nc.dram_tensor("woutT", [DPC, NS], f32r, kind="ExternalInput")
    cmask = nc.dram_tensor("cmask", [DH, 4 * 512 + 256], f32r, kind="ExternalInput")
    outp = nc.dram_tensor("outp", [TOK, NS], f32, kind="ExternalOutput")

    NT = TOK // 512   # 8 tok-tiles of 512
    NK = NS // 128    # 16 contraction chunks

    with tile.TileContext(nc) as tc:
        with (
            tc.tile_pool(name="constp", bufs=1) as constp,
            tc.tile_pool(name="vresp", bufs=1) as vresp,
            tc.tile_pool(name="dramp", bufs=1, space="DRAM") as dramp,
        ):
            cmask_sb = constp.tile([DH, 4 * 512 + 256], f32r)
            nc.sync.dma_start(out=cmask_sb, in_=cmask[:, :])
            identity = cmask_sb[:, 2048:2176]
            ones_col = cmask_sb[:, 2176:2177]
            ones_row = cmask_sb[0:1, 2176:2304]

            # V resident across phases 1-2: v_res[p, c, e] = V[c*128+p, e]
            v_res = vresp.tile([128, TOK // 128, DPC], f32r)

            # wvT staging through DRAM between phases 2 and 3
            wvn_d = dramp.tile([B * DPC, S], f32r)

            with tc.tile_pool(name="qkresp", bufs=1) as qkresp:
                # Q,K resident [e-block(q0,q1,k0,k1), tok]
                qk_res = qkresp.tile([128, 4, TOK], f32r)

                # ---------------- phase 1: QKV projection ----------------
                with (
                    tc.tile_pool(name="wp", bufs=1) as wp,
                    tc.tile_pool(name="xp", bufs=3) as xp,
                    tc.tile_pool(name="vstage", bufs=3) as vstage,
                    tc.tile_pool(name="qkv_ps", bufs=6, space="PSUM") as qkv_ps,
                    tc.tile_pool(name="tr_ps", bufs=2, space="PSUM") as tr_ps,
                ):
                    w_sb = wp.tile([128, NK, 6 * DH], f32r)
                    nc.sync.dma_start(
                        out=w_sb, in_=wT.rearrange("(c p) e -> p c e", p=128)
                    )

                    for a in range(NT):
                        pss = [
                            qkv_ps.tile([128, 512], f32, tag="qkv", name=f"qkv{m}")
                            for m in range(6)
                        ]
                        for half in range(2):
                            x_sb = xp.tile([128, NK // 2, 512], f32r, tag="x_sb")
                            nc.sync.dma_start(
                                out=x_sb,
                                in_=xT[
                                    1024 * half : 1024 * (half + 1),
                                    512 * a : 512 * (a + 1),
                                ].rearrange("(c p) t -> p c t", p=128),
                            )
                            for m in range(6):
                                for kc in range(NK // 2):
                                    kk = half * (NK // 2) + kc
                                    nc.tensor.matmul(
                                        pss[m],
                                        w_sb[:, kk, 128 * m : 128 * (m + 1)],
                                        x_sb[:, kc, :],
                                        start=(kk == 0),
                                        stop=(kk == NK - 1),
                                    )
                        for m in range(4):
                            # Q/K to resident SBUF in [e, tok] layout
                            nc.vector.tensor_copy(
                                out=qk_res[:, m, 512 * a : 512 * (a + 1)],
                                in_=pss[m],
                            )
                        for h in range(HPC):
                            # V: transpose [e,tok] -> [tok,e] into v_res
                            vs = vstage.tile([128, 512], f32r, tag="v")
                            nc.vector.tensor_copy(out=vs, in_=pss[4 + h])
                            for t in range(4):
                                tp = tr_ps.tile([128, 128], f32r, tag="tp")
                                nc.tensor.transpose(
                                    tp, vs[:, 128 * t : 128 * (t + 1)], identity
                                )
                                nc.vector.tensor_copy(
                                    out=v_res[
                                        :, 4 * a + t, 128 * h : 128 * (h + 1)
                                    ],
                                    in_=tp,
                                )

                # ---------------- phase 2: attention ----------------
                with (
                    tc.tile_pool(name="ptp", bufs=4) as ptp,
                    tc.tile_pool(name="zrp", bufs=2) as zrp,
                    tc.tile_pool(name="wvnp", bufs=2) as wvnp,
                    tc.tile_pool(name="sc_ps", bufs=2, space="PSUM") as sc_ps,
                    tc.tile_pool(name="wv_ps", bufs=2, space="PSUM") as wv_ps,
                    tc.tile_pool(name="z_ps", bufs=2, space="PSUM") as z_ps,
                    tc.tile_pool(name="zb_ps", bufs=2, space="PSUM") as zb_ps,
                ):
                    for b in range(B):
                        for h in range(HPC):
                            q_sb = qk_res[:, h, S * b : S * (b + 1)]
                            k_sb = qk_res[:, 2 + h, S * b : S * (b + 1)]
                            wvn = wvnp.tile([128, S], f32r, tag="wvn")
                            for ast in range(S // 512):
                                nj = 4 * ast + 4  # causal t-blocks
                                wv = wv_ps.tile([128, 512], f32, tag="wv")
                                z = z_ps.tile([1, 512], f32, tag="z")
                                for j in range(nj):
                                    sc = sc_ps.tile([128, 512], f32, tag="sc")
                                    nc.tensor.matmul(
                                        sc,
                                        k_sb[:, 128 * j : 128 * (j + 1)],
                                        q_sb[:, 512 * ast : 512 * (ast + 1)],
                                        start=True,
                                        stop=True,
                                    )
                                    pt = ptp.tile([128, 512], f32r, tag="pt")
                                    nc.scalar.activation(
                                        out=pt,
                                        in_=sc,
                                        func=mybir.ActivationFunctionType.Exp,
                                        scale=SCALE,
                                    )
                                    p = j - 4 * ast
                                    if p >= 0:
                                        nc.vector.tensor_mul(
                                            pt,
                                            pt,
                                            cmask_sb[:, 512 * p : 512 * (p + 1)],
                                        )
                                    nc.tensor.matmul(
                                        z,
                                        ones_col,
                                        pt,
                                        start=(j == 0),
                                        stop=(j == nj - 1),
                                    )
                                    nc.tensor.matmul(
                                        wv,
                                        v_res[
                                            :, 16 * b + j, 128 * h : 128 * (h + 1)
                                        ],
                                        pt,
                                        start=(j == 0),
                                        stop=(j == nj - 1),
                                    )
                                zr = zrp.tile([1, 512], f32r, tag="zr")
                                with nc.allow_low_precision(
                                    reason="f32r is bit-identical to f32"
                                ):
                                    nc.vector.reciprocal(out=zr, in_=z)
                                zb = zb_ps.tile([128, 512], f32, tag="zb")
                                nc.tensor.matmul(
                                    zb, ones_row, zr, start=True, stop=True
                                )
                                zbs = zrp.tile([128, 512], f32r, tag="zbs")
                                nc.vector.tensor_copy(out=zbs, in_=zb)
                                nc.vector.tensor_mul(
                                    wvn[:, 512 * ast : 512 * (ast + 1)], wv, zbs
                                )
                            nc.sync.dma_start(
                                out=wvn_d[
                                    (b * HPC + h) * DH : (b * HPC + h + 1) * DH, :
                                ],
                                in_=wvn,
                            )

            # ---------------- phase 3: output projection ----------------
            with (
                tc.tile_pool(name="woutp", bufs=1) as woutp,
                tc.tile_pool(name="wvlp", bufs=3) as wvlp,
                tc.tile_pool(name="ostage", bufs=3) as ostage,
                tc.tile_pool(name="o_ps", bufs=4, space="PSUM") as o_ps,
            ):
                wout_sb = woutp.tile([128, HPC, NS], f32r)
                nc.sync.dma_start(
                    out=wout_sb, in_=woutT.rearrange("(c p) e -> p c e", p=128)
                )
                for b in range(B):
                    wvl = wvlp.tile([128, HPC, S], f32r, tag="wvl")
                    nc.sync.dma_start(
                        out=wvl,
                        in_=wvn_d[b * DPC : (b + 1) * DPC, :].rearrange(
                            "(c p) t -> p c t", p=128
                        ),
                    )
                    for tk in range(S // 128):
                        ost = ostage.tile([128, NS], f32, tag="ost")
                        for n in range(NS // 512):
                            ops = o_ps.tile([128, 512], f32, tag="ops")
                            for h in range(HPC):
                                nc.tensor.matmul(
                                    ops,
                                    wvl[:, h, 128 * tk : 128 * (tk + 1)],
                                    wout_sb[:, h, 512 * n : 512 * (n + 1)],
                                    start=(h == 0),
                                    stop=(h == HPC - 1),
                                )
                            nc.vector.tensor_copy(
                                out=ost[:, 512 * n : 512 * (n + 1)], in_=ops
                            )
                        nc.sync.dma_start(
                            out=outp[
                                S * b + 128 * tk : S * b + 128 * (tk + 1), :
                            ],
                            in_=ost,
                        )

    nc.compile()
    return nc


def _causal_fastpath_ok(mask, cache_pos):
    if cache_pos.shape != (S,) or not np.array_equal(
        np.asarray(cache_pos), np.arange(S, dtype=np.int64).astype(cache_pos.dtype)
    ):
        return False
    m = np.asarray(mask).reshape(S, T)
    rows = np.arange(S)[:, None]
    cols = np.arange(T)[None, :]
    return np.array_equal(m, cols <= rows)


def _numpy_fallback(input_ids, mask, cache_pos, w_qkv, w_out, k_cache, v_cache):
    x = np.asarray(input_ids, dtype=np.float32)
    qkv = np.einsum("bsd,ed->bse", x, np.asarray(w_qkv, np.float32))
    q, k, v = np.split(qkv, 3, axis=-1)

    def heads(t):
        return t.reshape(B, S, H, DH).transpose(0, 2, 1, 3)

    q, k, v = heads(q), heads(k), heads(v)
    kf = np.array(k_cache, np.float32)
    vf = np.array(v_cache, np.float32)
    kf[:, :, np.asarray(cache_pos)] = k
    vf[:, :, np.asarray(cache_pos)] = v
    sc = np.einsum("bhsd,bhtd->bhst", q, kf) * SCALE
    sc = np.where(np.asarray(mask), sc, np.finfo(np.float32).min)
    sc = sc - sc.max(axis=-1, keepdims=True)
    p = np.exp(sc)
    p = p / p.sum(axis=-1, keepdims=True)
    wv = np.einsum("bhst,bhtd->bhsd", p, vf)
    wv = wv.transpose(0, 2, 1, 3).reshape(B, S, NS)
    return np.einsum("bsd,ed->bse", wv, np.asarray(w_out, np.float32))


def _build_cmask_host():
    # 4 multiplicative mask tiles [128, 512] laid side by side: tile p is
    # applied to scoresT block (t rows) against an s-tile of width 512 when
    # the t-block is the p-th 128-strip inside that s-tile.
    t = np.arange(128)[:, None]
    s = np.arange(512)[None, :]
    tiles = []
    for p in range(4):
        tiles.append(((s - 128 * p) >= t).astype(np.float32))
    # trailing constant blocks: [identity(128) | ones(128)]
    tiles.append(np.eye(128, dtype=np.float32))
    tiles.append(np.ones((128, 128), dtype=np.float32))
    return np.concatenate(tiles, axis=1)  # [128, 2304]


def _run_on_device(in_maps, trace=False):
    from concourse.bass_utils import run_bass_kernel_spmd

    if "nc" not in _CACHED:
        _CACHED["nc"] = _build_program()
    nc = _CACHED["nc"]
    return run_bass_kernel_spmd(
        nc, in_maps, core_ids=list(range(NCORES)), trace=trace
    )


def _prep_in_maps(input_ids, w_qkv, w_out):
    x2d = np.ascontiguousarray(
        np.asarray(input_ids, np.float32).reshape(TOK, NS).T
    )  # [NS, TOK]
    cm = _build_cmask_host()
    wq = np.asarray(w_qkv, np.float32)
    wo = np.asarray(w_out, np.float32)
    in_maps = []
    for c in range(NCORES):
        lo, hi = c * DPC, (c + 1) * DPC
        w_slice = np.concatenate(
            [wq[lo:hi], wq[NS + lo : NS + hi], wq[2 * NS + lo : 2 * NS + hi]],
            axis=0,
        )  # [768, NS] (q,k,v rows for this core's heads)
        wT_c = np.ascontiguousarray(w_slice.T)        # [NS, 768]
        woutT_c = np.ascontiguousarray(wo[:, lo:hi].T)  # [DPC, NS]
        in_maps.append({"xT": x2d, "wT": wT_c, "woutT": woutT_c, "cmask": cm})
    return in_maps


def kernel(input_ids, mask, cache_pos, w_qkv, w_out, k_cache, v_cache):
    if not _causal_fastpath_ok(mask, cache_pos):
        return _numpy_fallback(
            input_ids, mask, cache_pos, w_qkv, w_out, k_cache, v_cache
        )
    in_maps = _prep_in_maps(input_ids, w_qkv, w_out)
    res = _run_on_device(in_maps)
    out = np.zeros((TOK, NS), np.float32)
    for r in res.results:
        out += r["outp"]
    return out.reshape(B, S, NS)
